# revision 3
# baseline (speedup 1.0000x reference)
"""Basket embedding mean-pool (segment reduce) on 8 Trainium2 NeuronCores.

Data-parallel over batch (1024 -> 8 x 128, one batch row per SBUF partition).
Each core runs an identical Bass/Tile program:

  - masked item ids on device: invalid slots (m >= basket_len) -> V, a zero row
    appended to the table, so the plain sum over all M slots equals the masked
    sum (no mask multiply needed).
  - gather emb rows via GPSIMD indirect DMA, one offset per partition per
    instruction (the only indirect-DMA shape that is exact on this hardware),
    one instruction per (l, m) slot column -> dst tile [128, M, H] per l.
  - VectorE reduces the M axis, multiplies by 1/max(len,1), result DMA'd out.
"""

import numpy as np

from concourse import bacc, bass, mybir, tile
from concourse.bass_utils import run_bass_kernel_spmd

F32 = mybir.dt.float32
I32 = mybir.dt.int32
ALU = mybir.AluOpType

B, L, M, H, V = 1024, 50, 20, 64, 100000
N_CORES = 8
B_LOC = B // N_CORES


def build_nc():
    BIG = 1 << 20

    nc = bacc.Bacc("TRN2", target_bir_lowering=False, debug=False)
    ids_d = nc.dram_tensor("item_ids", [B_LOC, L, M], I32, kind="ExternalInput")
    lens_d = nc.dram_tensor("basket_lens", [B_LOC, L], I32, kind="ExternalInput")
    emb_d = nc.dram_tensor("emb", [V + 1, H], F32, kind="ExternalInput")
    out_d = nc.dram_tensor("out", [B_LOC, L, H], F32, kind="ExternalOutput")

    with tile.TileContext(nc) as tc:
        with tc.tile_pool(name="main", bufs=1) as pool, tc.tile_pool(
            name="gat", bufs=3
        ) as gpool:
            ids_t = pool.tile([128, L, M], I32, name="ids_t")
            nc.sync.dma_start(out=ids_t[:], in_=ids_d.ap())
            lens_t = pool.tile([128, L], I32, name="lens_t")
            nc.sync.dma_start(out=lens_t[:], in_=lens_d.ap())

            iota_t = pool.tile([128, M], I32, name="iota_t")
            nc.gpsimd.iota(iota_t[:], [[1, M]], base=0, channel_multiplier=0)

            # midx = min(ids + BIG * (iota_m >= len), V):
            # valid slots keep their id, invalid slots point at the zero row V.
            midx_t = pool.tile([128, L, M], I32, name="midx_t")
            nc.vector.tensor_tensor(
                out=midx_t[:],
                in0=iota_t[:, None, :].to_broadcast([128, L, M]),
                in1=lens_t[:, :, None].to_broadcast([128, L, M]),
                op=ALU.is_ge,
            )
            nc.vector.tensor_scalar(
                out=midx_t[:], in0=midx_t[:], scalar1=BIG, scalar2=None, op0=ALU.mult
            )
            nc.vector.tensor_tensor(
                out=midx_t[:], in0=midx_t[:], in1=ids_t[:], op=ALU.add
            )
            nc.vector.tensor_scalar(
                out=midx_t[:], in0=midx_t[:], scalar1=V, scalar2=None, op0=ALU.min
            )

            # 1 / max(len, 1)
            maxlen_t = pool.tile([128, L], I32, name="maxlen_t")
            nc.vector.tensor_scalar(
                out=maxlen_t[:], in0=lens_t[:], scalar1=1, scalar2=None, op0=ALU.max
            )
            lens_f = pool.tile([128, L], F32, name="lens_f")
            nc.vector.tensor_copy(out=lens_f[:], in_=maxlen_t[:])
            recip_t = pool.tile([128, L], F32, name="recip_t")
            nc.vector.reciprocal(out=recip_t[:], in_=lens_f[:])

            out_t = pool.tile([128, L, H], F32, name="out_t")

            for l in range(L):
                dst_t = gpool.tile([128, M, H], F32, name="dst_t", tag="dst")
                for m in range(M):
                    nc.gpsimd.indirect_dma_start(
                        out=dst_t[:, m, :],
                        out_offset=None,
                        in_=emb_d.ap(),
                        in_offset=bass.IndirectOffsetOnAxis(
                            ap=midx_t[:, l, m : m + 1], axis=0
                        ),
                        compute_op=ALU.bypass,
                    )
                # sum over the M slots -> [128, H]
                nc.vector.tensor_reduce(
                    out=out_t[:, l, :],
                    in_=dst_t[:].rearrange("p m h -> p h m"),
                    axis=mybir.AxisListType.X,
                    op=ALU.add,
                )

            nc.vector.tensor_tensor(
                out=out_t[:],
                in0=out_t[:],
                in1=recip_t[:, :, None].to_broadcast([128, L, H]),
                op=ALU.mult,
            )
            nc.sync.dma_start(out=out_d.ap(), in_=out_t[:])

    nc.compile()
    return nc


def run(item_ids, basket_lens, emb, trace=False):
    item_ids = np.ascontiguousarray(item_ids, dtype=np.int32)
    basket_lens = np.ascontiguousarray(basket_lens, dtype=np.int32)
    emb = np.ascontiguousarray(emb, dtype=np.float32)
    emb_p = np.concatenate([emb, np.zeros((1, H), np.float32)], axis=0)

    nc = build_nc()
    in_maps = [
        {
            "item_ids": item_ids[c * B_LOC : (c + 1) * B_LOC],
            "basket_lens": basket_lens[c * B_LOC : (c + 1) * B_LOC],
            "emb": emb_p,
        }
        for c in range(N_CORES)
    ]
    res = run_bass_kernel_spmd(nc, in_maps, core_ids=list(range(N_CORES)), trace=trace)
    out = np.concatenate(
        [np.asarray(r["out"]).reshape(B_LOC, L, H) for r in res.results], axis=0
    )
    return out, res


def kernel(item_ids, basket_lens, emb):
    out, _ = run(item_ids, basket_lens, emb)
    return out


# revision 5
# speedup vs baseline: 1.3789x; 1.3789x over previous
"""Basket embedding mean-pool (segment reduce) on 8 Trainium2 NeuronCores.

Data-parallel over batch (1024 -> 8 x 128, one batch row per SBUF partition).
Each core runs an identical Bass/Tile program:

  - masked item ids on device: invalid slots (m >= basket_len) -> V, a zero row
    appended to the table, so the plain sum over all M slots equals the masked
    sum (no mask multiply needed).
  - gather emb rows via GPSIMD indirect DMA, one offset per partition per
    instruction (the only indirect-DMA shape that is exact on this hardware),
    one instruction per (l, m) slot column -> dst tile [128, M, H] per l.
  - VectorE reduces the M axis, multiplies by 1/max(len,1), result DMA'd out.
"""

import numpy as np

from concourse import bacc, bass, mybir, tile
from concourse.bass_utils import run_bass_kernel_spmd

F32 = mybir.dt.float32
I32 = mybir.dt.int32
ALU = mybir.AluOpType

B, L, M, H, V = 1024, 50, 20, 64, 100000
N_CORES = 8
B_LOC = B // N_CORES


def build_nc(num_swdge_queues=4):
    BIG = 1 << 20

    nc = bacc.Bacc("TRN2", target_bir_lowering=False, debug=False,
                   num_swdge_queues=num_swdge_queues)
    ids_d = nc.dram_tensor("item_ids", [B_LOC, L, M], I32, kind="ExternalInput")
    lens_d = nc.dram_tensor("basket_lens", [B_LOC, L], I32, kind="ExternalInput")
    emb_d = nc.dram_tensor("emb", [V + 1, H], F32, kind="ExternalInput")
    out_d = nc.dram_tensor("out", [B_LOC, L, H], F32, kind="ExternalOutput")

    with tile.TileContext(nc) as tc:
        with tc.tile_pool(name="main", bufs=1) as pool, tc.tile_pool(
            name="gat", bufs=3
        ) as gpool:
            ids_t = pool.tile([128, L, M], I32, name="ids_t")
            nc.sync.dma_start(out=ids_t[:], in_=ids_d.ap())
            lens_t = pool.tile([128, L], I32, name="lens_t")
            nc.sync.dma_start(out=lens_t[:], in_=lens_d.ap())

            iota_t = pool.tile([128, M], I32, name="iota_t")
            nc.gpsimd.iota(iota_t[:], [[1, M]], base=0, channel_multiplier=0)

            # midx = min(ids + BIG * (iota_m >= len), V):
            # valid slots keep their id, invalid slots point at the zero row V.
            midx_t = pool.tile([128, L, M], I32, name="midx_t")
            nc.vector.tensor_tensor(
                out=midx_t[:],
                in0=iota_t[:, None, :].to_broadcast([128, L, M]),
                in1=lens_t[:, :, None].to_broadcast([128, L, M]),
                op=ALU.is_ge,
            )
            nc.vector.tensor_scalar(
                out=midx_t[:], in0=midx_t[:], scalar1=BIG, scalar2=None, op0=ALU.mult
            )
            nc.vector.tensor_tensor(
                out=midx_t[:], in0=midx_t[:], in1=ids_t[:], op=ALU.add
            )
            nc.vector.tensor_scalar(
                out=midx_t[:], in0=midx_t[:], scalar1=V, scalar2=None, op0=ALU.min
            )

            # 1 / max(len, 1)
            maxlen_t = pool.tile([128, L], I32, name="maxlen_t")
            nc.vector.tensor_scalar(
                out=maxlen_t[:], in0=lens_t[:], scalar1=1, scalar2=None, op0=ALU.max
            )
            lens_f = pool.tile([128, L], F32, name="lens_f")
            nc.vector.tensor_copy(out=lens_f[:], in_=maxlen_t[:])
            recip_t = pool.tile([128, L], F32, name="recip_t")
            nc.vector.reciprocal(out=recip_t[:], in_=lens_f[:])

            out_t = pool.tile([128, L, H], F32, name="out_t")

            for l in range(L):
                dst_t = gpool.tile([128, M, H], F32, name="dst_t", tag="dst")
                for m in range(M):
                    nc.gpsimd.indirect_dma_start(
                        out=dst_t[:, m, :],
                        out_offset=None,
                        in_=emb_d.ap(),
                        in_offset=bass.IndirectOffsetOnAxis(
                            ap=midx_t[:, l, m : m + 1], axis=0
                        ),
                        compute_op=ALU.bypass,
                    )
                # sum over the M slots -> [128, H]
                nc.vector.tensor_reduce(
                    out=out_t[:, l, :],
                    in_=dst_t[:].rearrange("p m h -> p h m"),
                    axis=mybir.AxisListType.X,
                    op=ALU.add,
                )

            nc.vector.tensor_tensor(
                out=out_t[:],
                in0=out_t[:],
                in1=recip_t[:, :, None].to_broadcast([128, L, H]),
                op=ALU.mult,
            )
            nc.sync.dma_start(out=out_d.ap(), in_=out_t[:])

    nc.compile()
    return nc


def run(item_ids, basket_lens, emb, trace=False):
    item_ids = np.ascontiguousarray(item_ids, dtype=np.int32)
    basket_lens = np.ascontiguousarray(basket_lens, dtype=np.int32)
    emb = np.ascontiguousarray(emb, dtype=np.float32)
    emb_p = np.concatenate([emb, np.zeros((1, H), np.float32)], axis=0)

    nc = build_nc()
    in_maps = [
        {
            "item_ids": item_ids[c * B_LOC : (c + 1) * B_LOC],
            "basket_lens": basket_lens[c * B_LOC : (c + 1) * B_LOC],
            "emb": emb_p,
        }
        for c in range(N_CORES)
    ]
    res = run_bass_kernel_spmd(nc, in_maps, core_ids=list(range(N_CORES)), trace=trace)
    out = np.concatenate(
        [np.asarray(r["out"]).reshape(B_LOC, L, H) for r in res.results], axis=0
    )
    return out, res


def kernel(item_ids, basket_lens, emb):
    out, _ = run(item_ids, basket_lens, emb)
    return out
